# revision 1
# baseline (speedup 1.0000x reference)
"""BinaryVQEncoder TRN2 kernel.

reference:
    z_e = x @ W + b                         [B, L]   (L = OUT_DIM, d = 1)
    dist[b,l,j] = (z_e[b,l] - emb[l,j,0])^2
    indices = argmin_j dist                 [B, L] int32
    quantized[b,l] = emb[l, indices[b,l], 0]
    quantized_st = z_e + (quantized - z_e)
    returns (indices, embedding, quantized_st, z_e)

Strategy: data-parallel over B across 8 cores. Per core computes
z^T [L, B_loc] with the tensor engine (stationary = W tile, moving = x^T),
then the VQ part as per-partition scalar ops (codebook params are
per-l = per-partition after the transpose):
    index = (c*z > t)  where  c = sign(e1-e0), t = c*(e0+e1)/2
    q     = index * (e1-e0) + e0
    qst   = z + (q - z)
Host transposes x and un-transposes the outputs.

Matmul scheme selected by MODE:
  f32   - native fp32 (4 cycles/row on PE)
  f32r  - 1-pass float32r (1 cycle/row, inputs truncated to ~13-bit significand)
  f16x3 - hi/lo fp16 split, 3 passes (1 cycle/row each), ~fp32 accuracy:
            z = xh@Wh + 2^-11 * (xh@Ws + xs@Wh)
          with xh = fp16(x), xs = fp16((x - xh) * 2^11)   (same for W)
"""

import os
import numpy as np

import concourse.bass as bass
import concourse.mybir as mybir
import concourse.tile as tile
from concourse import bacc
from concourse.bass_utils import run_bass_kernel_spmd

MODE = os.environ.get("VQ_KERNEL_MODE", "f16x3")

B, IN_DIM, OUT_DIM = 8192, 4096, 4096
NCORES = 8
BL = B // NCORES          # batch per core
P = 128
NB = 512                  # moving free size per matmul (one PSUM bank of fp32)
KO = IN_DIM // P          # 32 contraction chunks
OO = OUT_DIM // P         # 32 output tiles
BH = BL // NB             # 2 batch halves
F32 = mybir.dt.float32
ALU = mybir.AluOpType
LO_SCALE = 2.0 ** -11

_CACHE = {}


def _build(mode):
    nc = bacc.Bacc("TRN2", target_bir_lowering=False, debug=False)

    if mode == "f16x3":
        mdt = mybir.dt.float16
        xh_d = nc.dram_tensor("xhT", [IN_DIM, BL], mdt, kind="ExternalInput")
        xs_d = nc.dram_tensor("xsT", [IN_DIM, BL], mdt, kind="ExternalInput")
        W_d = nc.dram_tensor("Wp", [IN_DIM, 2, OUT_DIM], mdt, kind="ExternalInput")
        n_groups, OGH = 2, 1
    else:
        mdt = F32 if mode == "f32" else mybir.dt.float32r
        x_d = nc.dram_tensor("xT", [IN_DIM, BL], mdt, kind="ExternalInput")
        W_d = nc.dram_tensor("W", [IN_DIM, OUT_DIM], mdt, kind="ExternalInput")
        n_groups, OGH = 1, 2

    params_d = nc.dram_tensor("params", [P, 5, OO], F32, kind="ExternalInput")
    zT_d = nc.dram_tensor("zT", [OUT_DIM, BL], F32, kind="ExternalOutput")
    qstT_d = nc.dram_tensor("qstT", [OUT_DIM, BL], F32, kind="ExternalOutput")
    idxT_d = nc.dram_tensor("idxT", [OUT_DIM, BL], mybir.dt.uint8, kind="ExternalOutput")

    Ident = mybir.ActivationFunctionType.Identity

    with tile.TileContext(nc) as tc:
        with (
            tc.tile_pool(name="xpool", bufs=1) as xpool,
            tc.tile_pool(name="ppool", bufs=1) as ppool,
            tc.tile_pool(name="wpool", bufs=4) as wpool,
            tc.tile_pool(name="zpool", bufs=3) as zpool,
            tc.tile_pool(name="vpool", bufs=3) as vpool,
            tc.tile_pool(name="ipool", bufs=3) as ipool,
            tc.tile_pool(name="psum", bufs=2, space="PSUM") as psum_pool,
        ):
            params_sb = ppool.tile([P, 5, OO], F32)
            nc.sync.dma_start(params_sb[:], params_d.ap())

            # x resident in SBUF, chunked along the contraction dim
            if mode == "f16x3":
                xh_sb = xpool.tile([P, KO * BL], mdt)
                xs_sb = xpool.tile([P, KO * BL], mdt)
                xh_r = xh_d.ap().rearrange("(ko p) b -> ko p b", p=P)
                xs_r = xs_d.ap().rearrange("(ko p) b -> ko p b", p=P)
                for ko in range(KO):
                    nc.sync.dma_start(xh_sb[:, ko * BL:(ko + 1) * BL], xh_r[ko])
                    nc.sync.dma_start(xs_sb[:, ko * BL:(ko + 1) * BL], xs_r[ko])
            else:
                x_sb = xpool.tile([P, KO * BL], mdt)
                x_r = x_d.ap().rearrange("(ko p) b -> ko p b", p=P)
                for ko in range(KO):
                    nc.sync.dma_start(x_sb[:, ko * BL:(ko + 1) * BL], x_r[ko])

            def vq_tail(z, o, bh):
                """z [P, NB] holds z_e^T for o-tile `o`, batch half `bh`."""
                c_ap = params_sb[:, 1, o:o + 1]
                t_ap = params_sb[:, 2, o:o + 1]
                s_ap = params_sb[:, 3, o:o + 1]
                e0_ap = params_sb[:, 4, o:o + 1]
                rs = slice(o * P, (o + 1) * P)
                cs = slice(bh * NB, (bh + 1) * NB)
                nc.sync.dma_start(zT_d.ap()[rs, cs], z[:])
                mask = vpool.tile([P, NB], F32, name="mask")
                nc.vector.tensor_scalar(mask[:], z[:], c_ap, t_ap,
                                        op0=ALU.mult, op1=ALU.is_gt)
                q = vpool.tile([P, NB], F32, name="q")
                nc.vector.tensor_scalar(q[:], mask[:], s_ap, e0_ap,
                                        op0=ALU.mult, op1=ALU.add)
                tq = vpool.tile([P, NB], F32, name="tq")
                nc.vector.tensor_tensor(tq[:], q[:], z[:], op=ALU.subtract)
                qst = vpool.tile([P, NB], F32, name="qst")
                nc.vector.tensor_tensor(qst[:], z[:], tq[:], op=ALU.add)
                nc.sync.dma_start(qstT_d.ap()[rs, cs], qst[:])
                idx = ipool.tile([P, NB], mybir.dt.uint8, name="idx")
                nc.gpsimd.tensor_copy(idx[:], mask[:])
                nc.sync.dma_start(idxT_d.ap()[rs, cs], idx[:])

            for og in range(OO // OGH):
                # psum[ol][g][bh]
                ps = [[[psum_pool.tile([P, NB], F32, name=f"ps{ol}{g}{bh}",
                                       tag=f"ps{ol}{g}{bh}")
                        for bh in range(BH)] for g in range(n_groups)]
                      for ol in range(OGH)]

                for k in range(KO):
                    if mode == "f16x3":
                        o = og
                        wt = wpool.tile([P, 2, P], mdt, name="wt")
                        nc.sync.dma_start(
                            wt[:],
                            W_d.ap()[k * P:(k + 1) * P, :, o * P:(o + 1) * P])
                        wh = wt[:, 0, :]
                        ws = wt[:, 1, :]
                        for bh in range(BH):
                            nc.tensor.matmul(
                                ps[0][0][bh][:], wh,
                                xh_sb[:, k * BL + bh * NB: k * BL + (bh + 1) * NB],
                                start=(k == 0), stop=(k == KO - 1))
                        for bh in range(BH):
                            nc.tensor.matmul(
                                ps[0][1][bh][:], wh,
                                xs_sb[:, k * BL + bh * NB: k * BL + (bh + 1) * NB],
                                start=(k == 0), stop=False)
                        for bh in range(BH):
                            nc.tensor.matmul(
                                ps[0][1][bh][:], ws,
                                xh_sb[:, k * BL + bh * NB: k * BL + (bh + 1) * NB],
                                start=False, stop=(k == KO - 1))
                    else:
                        wt = wpool.tile([P, OGH * P], mdt, name="wt")
                        nc.sync.dma_start(
                            wt[:],
                            W_d.ap()[k * P:(k + 1) * P,
                                     og * OGH * P:(og + 1) * OGH * P])
                        for ol in range(OGH):
                            for bh in range(BH):
                                nc.tensor.matmul(
                                    ps[ol][0][bh][:], wt[:, ol * P:(ol + 1) * P],
                                    x_sb[:, k * BL + bh * NB: k * BL + (bh + 1) * NB],
                                    start=(k == 0), stop=(k == KO - 1))

                for ol in range(OGH):
                    o = og * OGH + ol
                    bias_ap = params_sb[:, 0, o:o + 1]
                    for bh in range(BH):
                        if n_groups == 2:
                            z1 = zpool.tile([P, NB], F32, name="z1")
                            nc.scalar.activation(z1[:], ps[ol][1][bh][:], Ident,
                                                 bias=bias_ap, scale=LO_SCALE)
                            z = zpool.tile([P, NB], F32, name="z")
                            nc.vector.tensor_tensor(z[:], z1[:], ps[ol][0][bh][:],
                                                    op=ALU.add)
                        else:
                            z = zpool.tile([P, NB], F32, name="z")
                            nc.scalar.activation(z[:], ps[ol][0][bh][:], Ident,
                                                 bias=bias_ap, scale=1.0)
                        vq_tail(z, o, bh)

    nc.compile()
    return nc


def _f16_flush(a):
    """fp16 cast with subnormals flushed to zero (mirror PE behaviour)."""
    h = a.astype(np.float16)
    h[np.abs(h) < 2.0 ** -14] = np.float16(0)
    return h


def kernel(x, W, b, embedding):
    x = np.asarray(x, dtype=np.float32)
    W = np.asarray(W, dtype=np.float32)
    b = np.asarray(b, dtype=np.float32)
    embedding = np.asarray(embedding, dtype=np.float32)

    if MODE not in _CACHE:
        _CACHE[MODE] = _build(MODE)
    nc = _CACHE[MODE]

    # per-l codebook params
    e0 = embedding[:, 0, 0]
    e1 = embedding[:, 1, 0]
    s = e1 - e0
    c = np.sign(s).astype(np.float32)
    m = (e0 + e1) * np.float32(0.5)
    t = np.where(s != 0, c * m, np.float32(1.0)).astype(np.float32)
    params = np.stack([b, c, t, s, e0])            # [5, OUT]
    params = np.ascontiguousarray(
        params.reshape(5, OO, P).transpose(2, 0, 1))  # [P, 5, OO]

    in_maps = []
    if MODE == "f16x3":
        Wh = _f16_flush(W)
        Ws = _f16_flush((W - Wh.astype(np.float32)) * np.float32(2.0 ** 11))
        Wp = np.ascontiguousarray(np.stack([Wh, Ws], axis=1))  # [IN, 2, OUT]
        xh = _f16_flush(x)
        xs = _f16_flush((x - xh.astype(np.float32)) * np.float32(2.0 ** 11))
        xhT = np.ascontiguousarray(xh.T)   # [IN, B]
        xsT = np.ascontiguousarray(xs.T)
        for i in range(NCORES):
            cs = slice(i * BL, (i + 1) * BL)
            in_maps.append({"xhT": np.ascontiguousarray(xhT[:, cs]),
                            "xsT": np.ascontiguousarray(xsT[:, cs]),
                            "Wp": Wp, "params": params})
    else:
        xT = np.ascontiguousarray(x.T)
        for i in range(NCORES):
            cs = slice(i * BL, (i + 1) * BL)
            in_maps.append({"xT": np.ascontiguousarray(xT[:, cs]),
                            "W": W, "params": params})

    trace = bool(os.environ.get("VQ_KERNEL_TRACE"))
    res = run_bass_kernel_spmd(nc, in_maps, core_ids=list(range(NCORES)),
                               trace=trace)
    if trace and res.exec_time_ns is not None:
        print(f"HW exec time: {res.exec_time_ns} ns")
        kernel.last_exec_time_ns = res.exec_time_ns
        kernel.last_trace = res.instructions_and_trace

    z_e = np.concatenate([r["zT"].T for r in res.results], axis=0)
    quantized_st = np.concatenate([r["qstT"].T for r in res.results], axis=0)
    indices = np.concatenate(
        [r["idxT"].T for r in res.results], axis=0).astype(np.int32)
    return indices, embedding, quantized_st, z_e


# revision 8
# speedup vs baseline: 1.3414x; 1.3414x over previous
"""BinaryVQEncoder TRN2 kernel.

reference:
    z_e = x @ W + b                         [B, L]   (L = OUT_DIM, d = 1)
    dist[b,l,j] = (z_e[b,l] - emb[l,j,0])^2
    indices = argmin_j dist                 [B, L] int32
    quantized[b,l] = emb[l, indices[b,l], 0]
    quantized_st = z_e + (quantized - z_e)
    returns (indices, embedding, quantized_st, z_e)

Strategy: data-parallel over B across 8 cores. Per core computes
z^T [L, B_loc] with the tensor engine (stationary = W tile, moving = x^T),
then the VQ part as per-partition scalar ops (codebook params are
per-l = per-partition after the transpose):
    index = (c*z > t)  where  c = sign(e1-e0), t = c*(e0+e1)/2
    q     = index * (e1-e0) + e0
    qst   = z + (q - z)
Host transposes x and un-transposes the outputs.

Matmul scheme selected by MODE:
  f32   - native fp32 (4 cycles/row on PE)
  f32r  - 1-pass float32r (1 cycle/row, inputs truncated to ~13-bit significand)
  f16x3 - hi/lo fp16 split, 3 passes (1 cycle/row each), ~fp32 accuracy:
            z = xh@Wh + 2^-11 * (xh@Ws + xs@Wh)
          with xh = fp16(x), xs = fp16((x - xh) * 2^11)   (same for W)
"""

import os
import numpy as np

import concourse.bass as bass
import concourse.mybir as mybir
import concourse.tile as tile
from concourse import bacc
from concourse.bass_utils import run_bass_kernel_spmd

MODE = os.environ.get("VQ_KERNEL_MODE", "f16x3")

B, IN_DIM, OUT_DIM = 8192, 4096, 4096
NCORES = 8
BL = B // NCORES          # batch per core
P = 128
NB = 512                  # moving free size per matmul (one PSUM bank of fp32)
KO = IN_DIM // P          # 32 contraction chunks
OO = OUT_DIM // P         # 32 output tiles
BH = BL // NB             # 2 batch halves
F32 = mybir.dt.float32
ALU = mybir.AluOpType
LO_SCALE = 2.0 ** -11

_CACHE = {}


def _build(mode):
    nc = bacc.Bacc("TRN2", target_bir_lowering=False, debug=False)

    if mode == "f16x3":
        mdt = mybir.dt.float16
        xh_d = nc.dram_tensor("xhT", [IN_DIM, BL], mdt, kind="ExternalInput")
        xs_d = nc.dram_tensor("xsT", [IN_DIM, BL], mdt, kind="ExternalInput")
        W_d = nc.dram_tensor("Wp", [IN_DIM, 2, OUT_DIM], mdt, kind="ExternalInput")
        n_groups, OGH = 2, 1
    else:
        mdt = F32 if mode == "f32" else mybir.dt.float32r
        x_d = nc.dram_tensor("xT", [IN_DIM, BL], mdt, kind="ExternalInput")
        W_d = nc.dram_tensor("W", [IN_DIM, OUT_DIM], mdt, kind="ExternalInput")
        n_groups, OGH = 1, 2

    params_d = nc.dram_tensor("params", [P, 5, OO], F32, kind="ExternalInput")
    zT_d = nc.dram_tensor("zT", [OUT_DIM, BL], F32, kind="ExternalOutput")
    qstT_d = nc.dram_tensor("qstT", [OUT_DIM, BL], F32, kind="ExternalOutput")
    idxT_d = nc.dram_tensor("idxT", [OUT_DIM, BL], mybir.dt.uint8, kind="ExternalOutput")

    Ident = mybir.ActivationFunctionType.Identity

    with tile.TileContext(nc) as tc:
        with (
            tc.tile_pool(name="xpool", bufs=1) as xpool,
            tc.tile_pool(name="ppool", bufs=1) as ppool,
            tc.tile_pool(name="wpool", bufs=6) as wpool,
            tc.tile_pool(name="zpool", bufs=3) as zpool,
            tc.tile_pool(name="vpool", bufs=3) as vpool,
            tc.tile_pool(name="ipool", bufs=3) as ipool,
            tc.tile_pool(name="psum", bufs=2, space="PSUM") as psum_pool,
        ):
            params_sb = ppool.tile([P, 5, OO], F32)
            nc.gpsimd.dma_start(params_sb[:], params_d.ap())

            # x resident in SBUF, chunked along the contraction dim
            if mode == "f16x3":
                xh_sb = xpool.tile([P, KO * BL], mdt)
                xs_sb = xpool.tile([P, KO * BL], mdt)
                xh_r = xh_d.ap().rearrange("(ko p) b -> ko p b", p=P)
                xs_r = xs_d.ap().rearrange("(ko p) b -> ko p b", p=P)
                for ko in range(KO):
                    nc.gpsimd.dma_start(xh_sb[:, ko * BL:(ko + 1) * BL], xh_r[ko])
                    nc.gpsimd.dma_start(xs_sb[:, ko * BL:(ko + 1) * BL], xs_r[ko])
            else:
                x_sb = xpool.tile([P, KO * BL], mdt)
                x_r = x_d.ap().rearrange("(ko p) b -> ko p b", p=P)
                for ko in range(KO):
                    nc.gpsimd.dma_start(x_sb[:, ko * BL:(ko + 1) * BL], x_r[ko])

            def vq_tail(z, o, bh):
                """z [P, NB] holds z_e^T for o-tile `o`, batch half `bh`."""
                c_ap = params_sb[:, 1, o:o + 1]
                t_ap = params_sb[:, 2, o:o + 1]
                s_ap = params_sb[:, 3, o:o + 1]
                e0_ap = params_sb[:, 4, o:o + 1]
                rs = slice(o * P, (o + 1) * P)
                cs = slice(bh * NB, (bh + 1) * NB)
                nc.scalar.dma_start(zT_d.ap()[rs, cs], z[:])
                mask = vpool.tile([P, NB], F32, name="mask")
                nc.vector.tensor_scalar(mask[:], z[:], c_ap, t_ap,
                                        op0=ALU.mult, op1=ALU.is_gt)
                q = vpool.tile([P, NB], F32, name="q")
                nc.vector.tensor_scalar(q[:], mask[:], s_ap, e0_ap,
                                        op0=ALU.mult, op1=ALU.add)
                tq = vpool.tile([P, NB], F32, name="tq")
                nc.vector.tensor_tensor(tq[:], q[:], z[:], op=ALU.subtract)
                qst = vpool.tile([P, NB], F32, name="qst")
                nc.vector.tensor_tensor(qst[:], z[:], tq[:], op=ALU.add)
                nc.scalar.dma_start(qstT_d.ap()[rs, cs], qst[:])
                idx = ipool.tile([P, NB], mybir.dt.uint8, name="idx")
                nc.gpsimd.tensor_copy(idx[:], mask[:])
                nc.gpsimd.dma_start(idxT_d.ap()[rs, cs], idx[:])

            for og in range(OO // OGH):
                # psum[ol][g][bh]
                ps = [[[psum_pool.tile([P, NB], F32, name=f"ps{ol}{g}{bh}",
                                       tag=f"ps{ol}{g}{bh}")
                        for bh in range(BH)] for g in range(n_groups)]
                      for ol in range(OGH)]

                for k in range(KO):
                    if mode == "f16x3":
                        o = og
                        wt = wpool.tile([P, 2, P], mdt, name="wt")
                        nc.sync.dma_start(
                            wt[:],
                            W_d.ap()[k * P:(k + 1) * P, :, o * P:(o + 1) * P])
                        wh = wt[:, 0, :]
                        ws = wt[:, 1, :]
                        for bh in range(BH):
                            nc.tensor.matmul(
                                ps[0][0][bh][:], wh,
                                xh_sb[:, k * BL + bh * NB: k * BL + (bh + 1) * NB],
                                start=(k == 0), stop=(k == KO - 1))
                        for bh in range(BH):
                            nc.tensor.matmul(
                                ps[0][1][bh][:], wh,
                                xs_sb[:, k * BL + bh * NB: k * BL + (bh + 1) * NB],
                                start=(k == 0), stop=False)
                        for bh in range(BH):
                            nc.tensor.matmul(
                                ps[0][1][bh][:], ws,
                                xh_sb[:, k * BL + bh * NB: k * BL + (bh + 1) * NB],
                                start=False, stop=(k == KO - 1))
                    else:
                        wt = wpool.tile([P, OGH * P], mdt, name="wt")
                        nc.sync.dma_start(
                            wt[:],
                            W_d.ap()[k * P:(k + 1) * P,
                                     og * OGH * P:(og + 1) * OGH * P])
                        for ol in range(OGH):
                            for bh in range(BH):
                                nc.tensor.matmul(
                                    ps[ol][0][bh][:], wt[:, ol * P:(ol + 1) * P],
                                    x_sb[:, k * BL + bh * NB: k * BL + (bh + 1) * NB],
                                    start=(k == 0), stop=(k == KO - 1))

                for ol in range(OGH):
                    o = og * OGH + ol
                    bias_ap = params_sb[:, 0, o:o + 1]
                    for bh in range(BH):
                        if n_groups == 2:
                            z1 = zpool.tile([P, NB], F32, name="z1")
                            nc.scalar.activation(z1[:], ps[ol][1][bh][:], Ident,
                                                 bias=bias_ap, scale=LO_SCALE)
                            z = zpool.tile([P, NB], F32, name="z")
                            nc.vector.tensor_tensor(z[:], z1[:], ps[ol][0][bh][:],
                                                    op=ALU.add)
                        else:
                            z = zpool.tile([P, NB], F32, name="z")
                            nc.scalar.activation(z[:], ps[ol][0][bh][:], Ident,
                                                 bias=bias_ap, scale=1.0)
                        vq_tail(z, o, bh)

    nc.compile()
    return nc


def _f16_flush(a):
    """fp16 cast with subnormals flushed to zero (mirror PE behaviour)."""
    h = a.astype(np.float16)
    h[np.abs(h) < 2.0 ** -14] = np.float16(0)
    return h


def kernel(x, W, b, embedding):
    x = np.asarray(x, dtype=np.float32)
    W = np.asarray(W, dtype=np.float32)
    b = np.asarray(b, dtype=np.float32)
    embedding = np.asarray(embedding, dtype=np.float32)

    if MODE not in _CACHE:
        _CACHE[MODE] = _build(MODE)
    nc = _CACHE[MODE]

    # per-l codebook params
    e0 = embedding[:, 0, 0]
    e1 = embedding[:, 1, 0]
    s = e1 - e0
    c = np.sign(s).astype(np.float32)
    m = (e0 + e1) * np.float32(0.5)
    t = np.where(s != 0, c * m, np.float32(1.0)).astype(np.float32)
    params = np.stack([b, c, t, s, e0])            # [5, OUT]
    params = np.ascontiguousarray(
        params.reshape(5, OO, P).transpose(2, 0, 1))  # [P, 5, OO]

    in_maps = []
    if MODE == "f16x3":
        Wh = _f16_flush(W)
        Ws = _f16_flush((W - Wh.astype(np.float32)) * np.float32(2.0 ** 11))
        Wp = np.ascontiguousarray(np.stack([Wh, Ws], axis=1))  # [IN, 2, OUT]
        xh = _f16_flush(x)
        xs = _f16_flush((x - xh.astype(np.float32)) * np.float32(2.0 ** 11))
        xhT = np.ascontiguousarray(xh.T)   # [IN, B]
        xsT = np.ascontiguousarray(xs.T)
        for i in range(NCORES):
            cs = slice(i * BL, (i + 1) * BL)
            in_maps.append({"xhT": np.ascontiguousarray(xhT[:, cs]),
                            "xsT": np.ascontiguousarray(xsT[:, cs]),
                            "Wp": Wp, "params": params})
    else:
        xT = np.ascontiguousarray(x.T)
        for i in range(NCORES):
            cs = slice(i * BL, (i + 1) * BL)
            in_maps.append({"xT": np.ascontiguousarray(xT[:, cs]),
                            "W": W, "params": params})

    trace = bool(os.environ.get("VQ_KERNEL_TRACE"))
    res = run_bass_kernel_spmd(nc, in_maps, core_ids=list(range(NCORES)),
                               trace=trace)
    if trace and res.exec_time_ns is not None:
        print(f"HW exec time: {res.exec_time_ns} ns")
        kernel.last_exec_time_ns = res.exec_time_ns
        kernel.last_trace = res.instructions_and_trace

    z_e = np.concatenate([r["zT"].T for r in res.results], axis=0)
    quantized_st = np.concatenate([r["qstT"].T for r in res.results], axis=0)
    indices = np.concatenate(
        [r["idxT"].T for r in res.results], axis=0).astype(np.int32)
    return indices, embedding, quantized_st, z_e


# revision 11
# speedup vs baseline: 1.5485x; 1.1544x over previous
"""BinaryVQEncoder TRN2 kernel.

reference:
    z_e = x @ W + b                         [B, L]   (L = OUT_DIM, d = 1)
    dist[b,l,j] = (z_e[b,l] - emb[l,j,0])^2
    indices = argmin_j dist                 [B, L] int32
    quantized[b,l] = emb[l, indices[b,l], 0]
    quantized_st = z_e + (quantized - z_e)
    returns (indices, embedding, quantized_st, z_e)

Strategy: data-parallel over B across 8 cores. Per core computes
z^T [L, B_loc] with the tensor engine (stationary = W tile, moving = x^T),
then the VQ part as per-partition scalar ops (codebook params are
per-l = per-partition after the transpose):
    index = (c*z > t)  where  c = sign(e1-e0), t = c*(e0+e1)/2
    q     = index * (e1-e0) + e0
    qst   = z + (q - z)
Host transposes x and un-transposes the outputs.

Matmul scheme selected by MODE:
  f32   - native fp32 (4 cycles/row on PE)
  f32r  - 1-pass float32r (1 cycle/row, inputs truncated to ~13-bit significand)
  f16x3 - hi/lo fp16 split, 3 passes (1 cycle/row each), ~fp32 accuracy:
            z = xh@Wh + 2^-11 * (xh@Ws + xs@Wh)
          with xh = fp16(x), xs = fp16((x - xh) * 2^11)   (same for W)
"""

import os
import numpy as np

import concourse.bass as bass
import concourse.mybir as mybir
import concourse.tile as tile
from concourse import bacc
from concourse.bass_utils import run_bass_kernel_spmd

MODE = os.environ.get("VQ_KERNEL_MODE", "f16x3")

B, IN_DIM, OUT_DIM = 8192, 4096, 4096
NCORES = 8
BL = B // NCORES          # batch per core
P = 128
NB = 512                  # moving free size per matmul (one PSUM bank of fp32)
KO = IN_DIM // P          # 32 contraction chunks
OO = OUT_DIM // P         # 32 output tiles
BH = BL // NB             # 2 batch halves
F32 = mybir.dt.float32
ALU = mybir.AluOpType
LO_SCALE = 2.0 ** -11

_CACHE = {}


def _build(mode):
    nc = bacc.Bacc("TRN2", target_bir_lowering=False, debug=False)

    if mode == "f16x3":
        mdt = mybir.dt.float16
        xh_d = nc.dram_tensor("xhT", [IN_DIM, BL], mdt, kind="ExternalInput")
        xs_d = nc.dram_tensor("xsT", [IN_DIM, BL], mdt, kind="ExternalInput")
        W_d = nc.dram_tensor("Wp", [IN_DIM, 2, OUT_DIM], mdt, kind="ExternalInput")
        n_groups, OGH = 2, 1
    else:
        mdt = {"f32": F32, "f32r": mybir.dt.float32r,
               "f16": mybir.dt.float16}[mode]
        x_d = nc.dram_tensor("xT", [IN_DIM, BL], mdt, kind="ExternalInput")
        W_d = nc.dram_tensor("W", [IN_DIM, OUT_DIM], mdt, kind="ExternalInput")
        n_groups, OGH = 1, 2

    params_d = nc.dram_tensor("params", [P, 5, OO], F32, kind="ExternalInput")
    zT_d = nc.dram_tensor("zT", [OUT_DIM, BL], F32, kind="ExternalOutput")
    qstT_d = nc.dram_tensor("qstT", [OUT_DIM, BL], F32, kind="ExternalOutput")
    idxT_d = nc.dram_tensor("idxT", [OUT_DIM, BL], mybir.dt.uint8, kind="ExternalOutput")

    Ident = mybir.ActivationFunctionType.Identity

    with tile.TileContext(nc) as tc:
        with (
            tc.tile_pool(name="xpool", bufs=1) as xpool,
            tc.tile_pool(name="ppool", bufs=1) as ppool,
            tc.tile_pool(name="wpool", bufs=6) as wpool,
            tc.tile_pool(name="zpool", bufs=3) as zpool,
            tc.tile_pool(name="vpool", bufs=3) as vpool,
            tc.tile_pool(name="ipool", bufs=3) as ipool,
            tc.tile_pool(name="psum", bufs=2, space="PSUM") as psum_pool,
        ):
            params_sb = ppool.tile([P, 5, OO], F32)
            nc.gpsimd.dma_start(params_sb[:], params_d.ap())

            # x resident in SBUF, chunked along the contraction dim
            if mode == "f16x3":
                xh_sb = xpool.tile([P, KO * BL], mdt)
                xs_sb = xpool.tile([P, KO * BL], mdt)
                xh_r = xh_d.ap().rearrange("(ko p) b -> ko p b", p=P)
                xs_r = xs_d.ap().rearrange("(ko p) b -> ko p b", p=P)
                for ko in range(KO):
                    nc.gpsimd.dma_start(xh_sb[:, ko * BL:(ko + 1) * BL], xh_r[ko])
                    nc.gpsimd.dma_start(xs_sb[:, ko * BL:(ko + 1) * BL], xs_r[ko])
            else:
                x_sb = xpool.tile([P, KO * BL], mdt)
                x_r = x_d.ap().rearrange("(ko p) b -> ko p b", p=P)
                for ko in range(KO):
                    nc.gpsimd.dma_start(x_sb[:, ko * BL:(ko + 1) * BL], x_r[ko])

            def vq_tail(z, o, bh):
                """z [P, NB] holds z_e^T for o-tile `o`, batch half `bh`."""
                c_ap = params_sb[:, 1, o:o + 1]
                t_ap = params_sb[:, 2, o:o + 1]
                s_ap = params_sb[:, 3, o:o + 1]
                e0_ap = params_sb[:, 4, o:o + 1]
                rs = slice(o * P, (o + 1) * P)
                cs = slice(bh * NB, (bh + 1) * NB)
                nc.scalar.dma_start(zT_d.ap()[rs, cs], z[:])
                mask = vpool.tile([P, NB], F32, name="mask")
                nc.vector.tensor_scalar(mask[:], z[:], c_ap, t_ap,
                                        op0=ALU.mult, op1=ALU.is_gt)
                q = vpool.tile([P, NB], F32, name="q")
                nc.vector.tensor_scalar(q[:], mask[:], s_ap, e0_ap,
                                        op0=ALU.mult, op1=ALU.add)
                tq = vpool.tile([P, NB], F32, name="tq")
                nc.vector.tensor_tensor(tq[:], q[:], z[:], op=ALU.subtract)
                qst = vpool.tile([P, NB], F32, name="qst")
                nc.vector.tensor_tensor(qst[:], z[:], tq[:], op=ALU.add)
                nc.scalar.dma_start(qstT_d.ap()[rs, cs], qst[:])
                idx = ipool.tile([P, NB], mybir.dt.uint8, name="idx")
                nc.gpsimd.tensor_copy(idx[:], mask[:])
                nc.gpsimd.dma_start(idxT_d.ap()[rs, cs], idx[:])

            for og in range(OO // OGH):
                # psum[ol][g][bh]
                ps = [[[psum_pool.tile([P, NB], F32, name=f"ps{ol}{g}{bh}",
                                       tag=f"ps{ol}{g}{bh}")
                        for bh in range(BH)] for g in range(n_groups)]
                      for ol in range(OGH)]

                for k in range(KO):
                    if mode == "f16x3":
                        o = og
                        wt = wpool.tile([P, 2, P], mdt, name="wt")
                        nc.sync.dma_start(
                            wt[:],
                            W_d.ap()[k * P:(k + 1) * P, :, o * P:(o + 1) * P])
                        wh = wt[:, 0, :]
                        ws = wt[:, 1, :]
                        for bh in range(BH):
                            nc.tensor.matmul(
                                ps[0][0][bh][:], wh,
                                xh_sb[:, k * BL + bh * NB: k * BL + (bh + 1) * NB],
                                start=(k == 0), stop=(k == KO - 1))
                        for bh in range(BH):
                            nc.tensor.matmul(
                                ps[0][1][bh][:], wh,
                                xs_sb[:, k * BL + bh * NB: k * BL + (bh + 1) * NB],
                                start=(k == 0), stop=False)
                        for bh in range(BH):
                            nc.tensor.matmul(
                                ps[0][1][bh][:], ws,
                                xh_sb[:, k * BL + bh * NB: k * BL + (bh + 1) * NB],
                                start=False, stop=(k == KO - 1))
                    else:
                        wt = wpool.tile([P, OGH * P], mdt, name="wt")
                        nc.sync.dma_start(
                            wt[:],
                            W_d.ap()[k * P:(k + 1) * P,
                                     og * OGH * P:(og + 1) * OGH * P])
                        for ol in range(OGH):
                            for bh in range(BH):
                                nc.tensor.matmul(
                                    ps[ol][0][bh][:], wt[:, ol * P:(ol + 1) * P],
                                    x_sb[:, k * BL + bh * NB: k * BL + (bh + 1) * NB],
                                    start=(k == 0), stop=(k == KO - 1))

                for ol in range(OGH):
                    o = og * OGH + ol
                    bias_ap = params_sb[:, 0, o:o + 1]
                    for bh in range(BH):
                        if n_groups == 2:
                            z1 = zpool.tile([P, NB], F32, name="z1")
                            nc.scalar.activation(z1[:], ps[ol][1][bh][:], Ident,
                                                 bias=bias_ap, scale=LO_SCALE)
                            z = zpool.tile([P, NB], F32, name="z")
                            nc.vector.tensor_tensor(z[:], z1[:], ps[ol][0][bh][:],
                                                    op=ALU.add)
                        else:
                            z = zpool.tile([P, NB], F32, name="z")
                            nc.scalar.activation(z[:], ps[ol][0][bh][:], Ident,
                                                 bias=bias_ap, scale=1.0)
                        vq_tail(z, o, bh)

    nc.compile()
    return nc


def _f16_flush(a):
    """fp16 cast with subnormals flushed to zero (mirror PE behaviour)."""
    h = a.astype(np.float16)
    h[np.abs(h) < 2.0 ** -14] = np.float16(0)
    return h


def kernel(x, W, b, embedding):
    x = np.asarray(x, dtype=np.float32)
    W = np.asarray(W, dtype=np.float32)
    b = np.asarray(b, dtype=np.float32)
    embedding = np.asarray(embedding, dtype=np.float32)

    if MODE not in _CACHE:
        _CACHE[MODE] = _build(MODE)
    nc = _CACHE[MODE]

    # per-l codebook params
    e0 = embedding[:, 0, 0]
    e1 = embedding[:, 1, 0]
    s = e1 - e0
    c = np.sign(s).astype(np.float32)
    m = (e0 + e1) * np.float32(0.5)
    t = np.where(s != 0, c * m, np.float32(1.0)).astype(np.float32)
    params = np.stack([b, c, t, s, e0])            # [5, OUT]
    params = np.ascontiguousarray(
        params.reshape(5, OO, P).transpose(2, 0, 1))  # [P, 5, OO]

    in_maps = []
    if MODE == "f16x3":
        Wh = _f16_flush(W)
        Ws = _f16_flush((W - Wh.astype(np.float32)) * np.float32(2.0 ** 11))
        Wp = np.ascontiguousarray(np.stack([Wh, Ws], axis=1))  # [IN, 2, OUT]
        xh = _f16_flush(x)
        xs = _f16_flush((x - xh.astype(np.float32)) * np.float32(2.0 ** 11))
        xhT = np.ascontiguousarray(xh.T)   # [IN, B]
        xsT = np.ascontiguousarray(xs.T)
        for i in range(NCORES):
            cs = slice(i * BL, (i + 1) * BL)
            in_maps.append({"xhT": np.ascontiguousarray(xhT[:, cs]),
                            "xsT": np.ascontiguousarray(xsT[:, cs]),
                            "Wp": Wp, "params": params})
    else:
        if MODE == "f16":
            xT = np.ascontiguousarray(x.T.astype(np.float16))
            Wd = W.astype(np.float16)
        else:
            xT = np.ascontiguousarray(x.T)
            Wd = W
        for i in range(NCORES):
            cs = slice(i * BL, (i + 1) * BL)
            in_maps.append({"xT": np.ascontiguousarray(xT[:, cs]),
                            "W": Wd, "params": params})

    trace = bool(os.environ.get("VQ_KERNEL_TRACE"))
    res = run_bass_kernel_spmd(nc, in_maps, core_ids=list(range(NCORES)),
                               trace=trace)
    if trace and res.exec_time_ns is not None:
        print(f"HW exec time: {res.exec_time_ns} ns")
        kernel.last_exec_time_ns = res.exec_time_ns
        kernel.last_trace = res.instructions_and_trace

    z_e = np.concatenate([r["zT"].T for r in res.results], axis=0)
    quantized_st = np.concatenate([r["qstT"].T for r in res.results], axis=0)
    indices = np.concatenate(
        [r["idxT"].T for r in res.results], axis=0).astype(np.int32)

    # Boundary repair: the device z_e carries a small, bounded matmul error
    # (reduced-precision operands). Elements whose z_e sits within DELTA of
    # the codeword midpoint may have a flipped argmin; recompute those few
    # exactly from the original fp32 inputs. DELTA is ~4x the measured
    # absmax device-z error for the mode, so every possible flip is inside
    # the captured set.
    delta = {"f16": 3e-3, "f32r": 3e-3, "f32": 4e-5, "f16x3": 4e-5}[MODE]
    sel = np.abs(z_e - m[None, :]) <= np.float32(delta)
    bb, ll = np.nonzero(sel)
    if bb.size:
        CH = 65536
        for lo in range(0, bb.size, CH):
            bc = bb[lo:lo + CH]
            lc = ll[lo:lo + CH]
            zr = np.einsum("ik,ki->i", x[bc].astype(np.float64),
                           W[:, lc].astype(np.float64))
            zr32 = (zr + b[lc]).astype(np.float32)
            d0 = (zr32 - e0[lc]) ** 2
            d1 = (zr32 - e1[lc]) ** 2
            ind = (d1 < d0).astype(np.int32)
            qv = np.where(ind == 1, e1[lc], e0[lc]).astype(np.float32)
            indices[bc, lc] = ind
            quantized_st[bc, lc] = zr32 + (qv - zr32)
            z_e[bc, lc] = zr32
    if os.environ.get("VQ_KERNEL_DEBUG"):
        print(f"repair: {bb.size} elements ({bb.size / z_e.size:.2e})")

    return indices, embedding, quantized_st, z_e


# revision 13
# speedup vs baseline: 1.7767x; 1.1474x over previous
"""BinaryVQEncoder TRN2 kernel.

reference:
    z_e = x @ W + b                         [B, L]   (L = OUT_DIM, d = 1)
    dist[b,l,j] = (z_e[b,l] - emb[l,j,0])^2
    indices = argmin_j dist                 [B, L] int32
    quantized[b,l] = emb[l, indices[b,l], 0]
    quantized_st = z_e + (quantized - z_e)
    returns (indices, embedding, quantized_st, z_e)

Strategy: data-parallel over B across 8 cores. Per core computes
z^T [L, B_loc] with the tensor engine (stationary = W tile, moving = x^T),
then the VQ part as per-partition scalar ops (codebook params are
per-l = per-partition after the transpose):
    index = (c*z > t)  where  c = sign(e1-e0), t = c*(e0+e1)/2
    q     = index * (e1-e0) + e0
    qst   = z + (q - z)
Host transposes x and un-transposes the outputs.

Matmul scheme selected by MODE:
  f32   - native fp32 (4 cycles/row on PE)
  f32r  - 1-pass float32r (1 cycle/row, inputs truncated to ~13-bit significand)
  f16x3 - hi/lo fp16 split, 3 passes (1 cycle/row each), ~fp32 accuracy:
            z = xh@Wh + 2^-11 * (xh@Ws + xs@Wh)
          with xh = fp16(x), xs = fp16((x - xh) * 2^11)   (same for W)
"""

import os
import numpy as np

import concourse.bass as bass
import concourse.mybir as mybir
import concourse.tile as tile
from concourse import bacc
from concourse.bass_utils import run_bass_kernel_spmd

MODE = os.environ.get("VQ_KERNEL_MODE", "f16x3")

B, IN_DIM, OUT_DIM = 8192, 4096, 4096
NCORES = 8
BL = B // NCORES          # batch per core
P = 128
NB = 512                  # moving free size per matmul (one PSUM bank of fp32)
KO = IN_DIM // P          # 32 contraction chunks
OO = OUT_DIM // P         # 32 output tiles
BH = BL // NB             # 2 batch halves
F32 = mybir.dt.float32
ALU = mybir.AluOpType
LO_SCALE = 2.0 ** -11

_CACHE = {}


def _build(mode):
    nc = bacc.Bacc("TRN2", target_bir_lowering=False, debug=False)

    if mode == "f16x3":
        mdt = mybir.dt.float16
        xh_d = nc.dram_tensor("xhT", [IN_DIM, BL], mdt, kind="ExternalInput")
        xs_d = nc.dram_tensor("xsT", [IN_DIM, BL], mdt, kind="ExternalInput")
        W_d = nc.dram_tensor("Wp", [IN_DIM, 2, OUT_DIM], mdt, kind="ExternalInput")
        n_groups, OGH = 2, 1
    else:
        mdt = {"f32": F32, "f32r": mybir.dt.float32r,
               "f16": mybir.dt.float16}[mode]
        x_d = nc.dram_tensor("xT", [IN_DIM, BL], mdt, kind="ExternalInput")
        W_d = nc.dram_tensor("W", [IN_DIM, OUT_DIM], mdt, kind="ExternalInput")
        n_groups, OGH = 1, 2

    params_d = nc.dram_tensor("params", [P, 5, OO], F32, kind="ExternalInput")
    zT_d = nc.dram_tensor("zT", [OUT_DIM, BL], F32, kind="ExternalOutput")
    qstT_d = nc.dram_tensor("qstT", [OUT_DIM, BL], F32, kind="ExternalOutput")
    idxT_d = nc.dram_tensor("idxT", [OUT_DIM, BL], mybir.dt.uint8, kind="ExternalOutput")

    Ident = mybir.ActivationFunctionType.Identity

    with tile.TileContext(nc) as tc:
        with (
            tc.tile_pool(name="xpool", bufs=1) as xpool,
            tc.tile_pool(name="ppool", bufs=1) as ppool,
            tc.tile_pool(name="wpool", bufs=12) as wpool,
            tc.tile_pool(name="zpool", bufs=3) as zpool,
            tc.tile_pool(name="vpool", bufs=3) as vpool,
            tc.tile_pool(name="ipool", bufs=3) as ipool,
            tc.tile_pool(name="psum", bufs=2, space="PSUM") as psum_pool,
        ):
            params_sb = ppool.tile([P, 5, OO], F32)
            nc.gpsimd.dma_start(params_sb[:], params_d.ap())

            # x resident in SBUF, chunked along the contraction dim
            if mode == "f16x3":
                xh_sb = xpool.tile([P, KO * BL], mdt)
                xs_sb = xpool.tile([P, KO * BL], mdt)
                xh_r = xh_d.ap().rearrange("(ko p) b -> ko p b", p=P)
                xs_r = xs_d.ap().rearrange("(ko p) b -> ko p b", p=P)
                for ko in range(KO):
                    nc.gpsimd.dma_start(xh_sb[:, ko * BL:(ko + 1) * BL], xh_r[ko])
                    nc.gpsimd.dma_start(xs_sb[:, ko * BL:(ko + 1) * BL], xs_r[ko])
            else:
                x_sb = xpool.tile([P, KO * BL], mdt)
                x_r = x_d.ap().rearrange("(ko p) b -> ko p b", p=P)
                for ko in range(KO):
                    nc.gpsimd.dma_start(x_sb[:, ko * BL:(ko + 1) * BL], x_r[ko])

            def vq_tail(z, o, bh):
                """z [P, NB] holds z_e^T for o-tile `o`, batch half `bh`."""
                c_ap = params_sb[:, 1, o:o + 1]
                t_ap = params_sb[:, 2, o:o + 1]
                s_ap = params_sb[:, 3, o:o + 1]
                e0_ap = params_sb[:, 4, o:o + 1]
                rs = slice(o * P, (o + 1) * P)
                cs = slice(bh * NB, (bh + 1) * NB)
                nc.scalar.dma_start(zT_d.ap()[rs, cs], z[:])
                mask = vpool.tile([P, NB], F32, name="mask")
                nc.vector.tensor_scalar(mask[:], z[:], c_ap, t_ap,
                                        op0=ALU.mult, op1=ALU.is_gt)
                q = vpool.tile([P, NB], F32, name="q")
                nc.vector.tensor_scalar(q[:], mask[:], s_ap, e0_ap,
                                        op0=ALU.mult, op1=ALU.add)
                tq = vpool.tile([P, NB], F32, name="tq")
                nc.vector.tensor_tensor(tq[:], q[:], z[:], op=ALU.subtract)
                qst = vpool.tile([P, NB], F32, name="qst")
                nc.vector.tensor_tensor(qst[:], z[:], tq[:], op=ALU.add)
                nc.scalar.dma_start(qstT_d.ap()[rs, cs], qst[:])
                idx = ipool.tile([P, NB], mybir.dt.uint8, name="idx")
                nc.vector.tensor_copy(idx[:], mask[:])
                nc.scalar.dma_start(idxT_d.ap()[rs, cs], idx[:])

            for og in range(OO // OGH):
                # psum[ol][g][bh]
                ps = [[[psum_pool.tile([P, NB], F32, name=f"ps{ol}{g}{bh}",
                                       tag=f"ps{ol}{g}{bh}")
                        for bh in range(BH)] for g in range(n_groups)]
                      for ol in range(OGH)]

                for k in range(KO):
                    if mode == "f16x3":
                        o = og
                        wt = wpool.tile([P, 2, P], mdt, name="wt")
                        nc.sync.dma_start(
                            wt[:],
                            W_d.ap()[k * P:(k + 1) * P, :, o * P:(o + 1) * P])
                        wh = wt[:, 0, :]
                        ws = wt[:, 1, :]
                        for bh in range(BH):
                            nc.tensor.matmul(
                                ps[0][0][bh][:], wh,
                                xh_sb[:, k * BL + bh * NB: k * BL + (bh + 1) * NB],
                                start=(k == 0), stop=(k == KO - 1))
                        for bh in range(BH):
                            nc.tensor.matmul(
                                ps[0][1][bh][:], wh,
                                xs_sb[:, k * BL + bh * NB: k * BL + (bh + 1) * NB],
                                start=(k == 0), stop=False)
                        for bh in range(BH):
                            nc.tensor.matmul(
                                ps[0][1][bh][:], ws,
                                xh_sb[:, k * BL + bh * NB: k * BL + (bh + 1) * NB],
                                start=False, stop=(k == KO - 1))
                    else:
                        wt = wpool.tile([P, OGH * P], mdt, name="wt")
                        nc.sync.dma_start(
                            wt[:],
                            W_d.ap()[k * P:(k + 1) * P,
                                     og * OGH * P:(og + 1) * OGH * P])
                        for ol in range(OGH):
                            for bh in range(BH):
                                nc.tensor.matmul(
                                    ps[ol][0][bh][:], wt[:, ol * P:(ol + 1) * P],
                                    x_sb[:, k * BL + bh * NB: k * BL + (bh + 1) * NB],
                                    start=(k == 0), stop=(k == KO - 1))

                for ol in range(OGH):
                    o = og * OGH + ol
                    bias_ap = params_sb[:, 0, o:o + 1]
                    for bh in range(BH):
                        if n_groups == 2:
                            z1 = zpool.tile([P, NB], F32, name="z1")
                            nc.scalar.activation(z1[:], ps[ol][1][bh][:], Ident,
                                                 bias=bias_ap, scale=LO_SCALE)
                            z = zpool.tile([P, NB], F32, name="z")
                            nc.vector.tensor_tensor(z[:], z1[:], ps[ol][0][bh][:],
                                                    op=ALU.add)
                        else:
                            z = zpool.tile([P, NB], F32, name="z")
                            nc.scalar.activation(z[:], ps[ol][0][bh][:], Ident,
                                                 bias=bias_ap, scale=1.0)
                        vq_tail(z, o, bh)

    nc.compile()
    return nc


def _f16_flush(a):
    """fp16 cast with subnormals flushed to zero (mirror PE behaviour)."""
    h = a.astype(np.float16)
    h[np.abs(h) < 2.0 ** -14] = np.float16(0)
    return h


def kernel(x, W, b, embedding):
    x = np.asarray(x, dtype=np.float32)
    W = np.asarray(W, dtype=np.float32)
    b = np.asarray(b, dtype=np.float32)
    embedding = np.asarray(embedding, dtype=np.float32)

    if MODE not in _CACHE:
        _CACHE[MODE] = _build(MODE)
    nc = _CACHE[MODE]

    # per-l codebook params
    e0 = embedding[:, 0, 0]
    e1 = embedding[:, 1, 0]
    s = e1 - e0
    c = np.sign(s).astype(np.float32)
    m = (e0 + e1) * np.float32(0.5)
    t = np.where(s != 0, c * m, np.float32(1.0)).astype(np.float32)
    params = np.stack([b, c, t, s, e0])            # [5, OUT]
    params = np.ascontiguousarray(
        params.reshape(5, OO, P).transpose(2, 0, 1))  # [P, 5, OO]

    in_maps = []
    if MODE == "f16x3":
        Wh = _f16_flush(W)
        Ws = _f16_flush((W - Wh.astype(np.float32)) * np.float32(2.0 ** 11))
        Wp = np.ascontiguousarray(np.stack([Wh, Ws], axis=1))  # [IN, 2, OUT]
        xh = _f16_flush(x)
        xs = _f16_flush((x - xh.astype(np.float32)) * np.float32(2.0 ** 11))
        xhT = np.ascontiguousarray(xh.T)   # [IN, B]
        xsT = np.ascontiguousarray(xs.T)
        for i in range(NCORES):
            cs = slice(i * BL, (i + 1) * BL)
            in_maps.append({"xhT": np.ascontiguousarray(xhT[:, cs]),
                            "xsT": np.ascontiguousarray(xsT[:, cs]),
                            "Wp": Wp, "params": params})
    else:
        if MODE == "f16":
            xT = np.ascontiguousarray(x.T.astype(np.float16))
            Wd = W.astype(np.float16)
        else:
            xT = np.ascontiguousarray(x.T)
            Wd = W
        for i in range(NCORES):
            cs = slice(i * BL, (i + 1) * BL)
            in_maps.append({"xT": np.ascontiguousarray(xT[:, cs]),
                            "W": Wd, "params": params})

    trace = bool(os.environ.get("VQ_KERNEL_TRACE"))
    res = run_bass_kernel_spmd(nc, in_maps, core_ids=list(range(NCORES)),
                               trace=trace)
    if trace and res.exec_time_ns is not None:
        print(f"HW exec time: {res.exec_time_ns} ns")
        kernel.last_exec_time_ns = res.exec_time_ns
        kernel.last_trace = res.instructions_and_trace

    z_e = np.concatenate([r["zT"].T for r in res.results], axis=0)
    quantized_st = np.concatenate([r["qstT"].T for r in res.results], axis=0)
    indices = np.concatenate(
        [r["idxT"].T for r in res.results], axis=0).astype(np.int32)

    # Boundary repair: the device z_e carries a small, bounded matmul error
    # (reduced-precision operands). Elements whose z_e sits within DELTA of
    # the codeword midpoint may have a flipped argmin; recompute those few
    # exactly from the original fp32 inputs. DELTA is ~4x the measured
    # absmax device-z error for the mode, so every possible flip is inside
    # the captured set.
    delta = {"f16": 3e-3, "f32r": 3e-3, "f32": 4e-5, "f16x3": 4e-5}[MODE]
    sel = np.abs(z_e - m[None, :]) <= np.float32(delta)
    bb, ll = np.nonzero(sel)
    if bb.size:
        CH = 65536
        for lo in range(0, bb.size, CH):
            bc = bb[lo:lo + CH]
            lc = ll[lo:lo + CH]
            zr = np.einsum("ik,ki->i", x[bc].astype(np.float64),
                           W[:, lc].astype(np.float64))
            zr32 = (zr + b[lc]).astype(np.float32)
            d0 = (zr32 - e0[lc]) ** 2
            d1 = (zr32 - e1[lc]) ** 2
            ind = (d1 < d0).astype(np.int32)
            qv = np.where(ind == 1, e1[lc], e0[lc]).astype(np.float32)
            indices[bc, lc] = ind
            quantized_st[bc, lc] = zr32 + (qv - zr32)
            z_e[bc, lc] = zr32
    if os.environ.get("VQ_KERNEL_DEBUG"):
        print(f"repair: {bb.size} elements ({bb.size / z_e.size:.2e})")

    return indices, embedding, quantized_st, z_e
